# revision 36
# baseline (speedup 1.0000x reference)
"""Trainium2 Bass kernel for nn_MultiHeadCrossAttention_57638461112647.

Sharding: 8 cores = 2 batches x 4-way split over attention *keys* (and,
identically, over output tokens). The softmax in the reference is over the
*query* axis (axis=1), so with scores laid out (keys on partitions, queries on
free) the softmax denominator Z[k] is a free-axis row-sum, fully core-local.
The only cross-core exchange is one ReduceScatter of the attention output
partials x^T = V'^T E (48 x 4096) within each 4-core batch group.

Structure (bf16 operands everywhere, fp32 PSUM accumulation; matmuls split to
the 512-element ISA moving limit):
  - scoresT = (M^T y1k)^T y1 with M = scale*Wq@Wk^T folded on host: Q and K
    never materialize on device, scores read the relu'd y1 directly.
  - Per-k-tile software pipeline: scores(kt) [PE] -> exp(kt) [ACT] -> Z(kt)
    row-sum [DVE] -> V'(kt)=V/Z [DVE] -> xT(kt) accumulated in PSUM. Z is
    per-KEY (row sum over the free/query axis), so each k-tile's Z closes as
    soon as its exps retire; the xT and conv matmuls ride in the PE slack of
    the ACT-bound exp phase (the xT/conv emission trails the scores by one
    k-tile so the in-order PE queue never stalls on V').
  - PSUM budget (8x2KB banks): score double-buffer 2x(128,1024) [4] + xT
    accumulator for query chunk 0 (48,1024) [2] + conv accumulator (96,1024)
    [2]. Query chunks 1-3 of xT re-read E from SBUF right after the phase.
  - The last k-tile's Z is computed as 4 per-chunk partial reduces so it
    closes right behind the final exp instead of a full-row reduce.
  - The ReduceScatter measures ~free on this hardware (intra-chip 4-rank
    groups); only the small out-projection tail follows it.
  - Cross-rep pipelining: e/y1/kq/v/vp double-buffered so consecutive repeat
    bodies overlap (the benchmark measures steady-state initiation interval,
    which sits at the PE busy floor of ~43us/rep).
  - Own-quarter reuse: KQ/V/S1 derive from a pre-sliced own-chunk input.

Host-side prep (cheap): fold BN-as-affine + biases into the 1x1-conv weights,
fold the attention scale into Wq, fold the 3x3x3-conv bias through the
following 1x1 conv, add the (constant) 3D positional encodings, pad the conv
input with its halo, bf16-round everything, and slice per-core chunks.
"""
import numpy as np
import ml_dtypes
import jax
from jax.sharding import Mesh, PartitionSpec
from jax.experimental.shard_map import shard_map

import concourse.bass as bass
import concourse.mybir as mybir
import concourse.tile as tile
from concourse import bacc
from concourse import bass2jax
from concourse.bass2jax import _bass_exec_p, install_neuronx_cc_hook

F32 = mybir.dt.float32
BF16 = mybir.dt.bfloat16
FP8 = mybir.dt.float8e4
AF = mybir.ActivationFunctionType
ADD = mybir.AluOpType.add
MAX = mybir.AluOpType.max

B, Cy, Cs, D, H, W = 2, 96, 48, 16, 16, 16
N = D * H * W            # 4096 tokens
NC = 8                   # cores
G = 4                    # cores per batch
KC = N // G              # keys / output tokens per core = 1024
KT = KC // 128           # k-tiles per core = 8
QC = 1024                # free-dim chunk per bf16 matmul
NQ = N // QC             # query chunks = 4
EPS = 1e-5

_cache = {}


# ---------------------------------------------------------------- host prep
def _pe3d(C, x, y, z):
    """Transcription of reference.pe3d (incl. its quirky torch broadcasting)."""
    c = int(np.ceil(C / 3))
    inv_freq = (1.0 / (10000.0 ** (np.arange(0, c, 2, dtype=np.float32) / c))
                ).astype(np.float32)

    def emb(n):
        s = np.arange(n, dtype=np.float32)[:, None] * inv_freq[None, :]
        return np.concatenate([np.sin(s), np.cos(s)], axis=-1).astype(np.float32)

    out = np.zeros((x, y, z, 3 * c), np.float32)
    out[..., :c] = emb(x)[:, None, :]        # broadcasts against (y, z, c)
    out[..., c:2 * c] = emb(y)[:, None, :]
    out[..., 2 * c:3 * c] = emb(z)
    return np.ascontiguousarray(out[..., :C].transpose(3, 0, 1, 2))  # (C,x,y,z)


def _prepare(inputs):
    f = lambda a: np.ascontiguousarray(np.asarray(a, np.float32))
    bf = lambda a: np.ascontiguousarray(np.asarray(a, ml_dtypes.bfloat16))
    Y, S = f(inputs['Y']), f(inputs['S'])

    pe_s = _pe3d(Cs, D, H, W).reshape(Cs, N)
    pe_y = _pe3d(Cy, D, H, W).reshape(Cy, N)
    Scm = S.reshape(B, Cs, N) + pe_s[None]
    Ycm = Y.reshape(B, Cy, N) + pe_y[None]

    sb = lambda g: f(g) / np.sqrt(np.float32(1.0) + np.float32(EPS))

    def fold(w, b, g, be):
        s = sb(g)
        return f(w) * s[:, None], (f(b) * s + f(be)).astype(np.float32)

    WsF, bsF = fold(inputs['w_s'], inputs['b_s'], inputs['g_s'], inputs['be_s'])
    WyF, byF = fold(inputs['w_y'], inputs['b_y'], inputs['g_y'], inputs['be_y'])
    WoF, boF = fold(inputs['w_o'], inputs['b_o'], inputs['g_o'], inputs['be_o'])
    Wy2F, by2F = fold(inputs['w_y2'], inputs['b_y2'], inputs['g_y2'], inputs['be_y2'])
    by2FF = (Wy2F @ f(inputs['b3']) + by2F).astype(np.float32)

    c = np.ascontiguousarray
    # scoresT = (M^T y1k)^T y1 with M = scale * Wq Wk^T: Q and K never
    # materialize on device.
    M = bf(np.float32(Cs) ** np.float32(-0.5) * f(inputs['Wq']) @ f(inputs['Wk']).T)
    Wv = bf(inputs['Wv'])
    w3T = bf(f(inputs['w3']).reshape(Cy, Cy, 27).transpose(2, 1, 0))  # (27,96,96)
    BIAS = c(np.stack([byF, bsF, boF, by2FF], axis=1))                # (48,4)

    Ypad = np.zeros((B, Cy, D + 2, H + 2, W + 2), np.float32)
    Ypad[:, :, 1:-1, 1:-1, 1:-1] = Ycm.reshape(B, Cy, D, H, W)

    in_maps = []
    for core in range(NC):
        b, g = divmod(core, G)
        d0 = g * (D // G)
        in_maps.append(dict(
            Yb=bf(Ycm[b]),                                 # (96,4096) bf16
            Ybk=bf(Ycm[b, :, g * KC:(g + 1) * KC]),        # (96,1024) bf16
            Sk=c(Scm[b, :, g * KC:(g + 1) * KC]),          # (48,1024) f32
            Yslab=bf(Ypad[b, :, d0:d0 + 6, :, :]),         # (96,6,18,18) bf16
            WsT=bf(WsF.T), WyT=bf(WyF.T), WoT=bf(WoF.T), Wy2T=bf(Wy2F.T),
            M=M, Wv=Wv, w3T=w3T, BIAS=BIAS,
        ))
    return in_maps


# ---------------------------------------------------------------- bass build
def _build(repeat=1, ablate=()):
    """ablate: subset of {'rs','attn','conv','qkv'} — for timing bisection
    only (results become wrong)."""
    nc = bacc.Bacc("TRN2", target_bir_lowering=False, debug=False, num_devices=NC)

    Yb = nc.dram_tensor("Yb", [Cy, N], BF16, kind="ExternalInput")
    Sk = nc.dram_tensor("Sk", [Cs, KC], F32, kind="ExternalInput")
    Yslab = nc.dram_tensor("Yslab", [Cy, 6, 18, 18], BF16, kind="ExternalInput")
    WsT = nc.dram_tensor("WsT", [Cs, Cs], BF16, kind="ExternalInput")
    WyT = nc.dram_tensor("WyT", [Cy, Cs], BF16, kind="ExternalInput")
    WoT = nc.dram_tensor("WoT", [Cs, Cs], BF16, kind="ExternalInput")
    Wy2T = nc.dram_tensor("Wy2T", [Cy, Cs], BF16, kind="ExternalInput")
    M = nc.dram_tensor("M", [Cs, Cs], BF16, kind="ExternalInput")
    Wv = nc.dram_tensor("Wv", [Cs, Cs], BF16, kind="ExternalInput")
    w3T = nc.dram_tensor("w3T", [27, Cy, Cy], BF16, kind="ExternalInput")
    BIAS = nc.dram_tensor("BIAS", [Cs, 4], F32, kind="ExternalInput")
    OUT = nc.dram_tensor("OUT", [2 * Cs, KC], F32, kind="ExternalOutput")

    # The NEFF is identical across cores, so the own-chunk index g cannot be
    # baked in; the host passes the own-quarter slice as a separate input.
    Ybk = nc.dram_tensor("Ybk", [Cy, KC], BF16, kind="ExternalInput")

    def mm2(out, lhsT, rhs, start=True, stop=True):
        """matmul split into <=512-wide halves (ISA moving-operand limit)."""
        n = out.shape[-1]
        if n <= 512:
            nc.tensor.matmul(out, lhsT, rhs, start=start, stop=stop)
            return
        h = n // 2
        nc.tensor.matmul(out[:, 0:h], lhsT, rhs[:, 0:h], start=start, stop=stop)
        nc.tensor.matmul(out[:, h:n], lhsT, rhs[:, h:n], start=start, stop=stop)

    with tile.TileContext(nc) as tc:
        with (
            tc.tile_pool(name="const", bufs=1) as cp,
            tc.tile_pool(name="data", bufs=1) as dp,
            tc.tile_pool(name="data2", bufs=2) as dp2,
            tc.tile_pool(name="chunk", bufs=2) as chp,
            tc.tile_pool(name="psum", bufs=2, space="PSUM") as pp,
            tc.tile_pool(name="psacc", bufs=1, space="PSUM") as pa,
            tc.tile_pool(name="dram", bufs=1, space="DRAM") as dram,
        ):
            # ---- constants into SBUF
            def load_const(t, shape, dt=BF16):
                s = cp.tile(shape, dt, tag=t.name)
                nc.scalar.dma_start(s[:], t.ap())
                return s

            wy = load_const(WyT, [Cy, Cs])
            wm = load_const(M, [Cs, Cs])
            ws = load_const(WsT, [Cs, Cs])
            wv = load_const(Wv, [Cs, Cs])
            bias = load_const(BIAS, [Cs, 4], F32)
            wo = load_const(WoT, [Cs, Cs])
            wy2 = load_const(Wy2T, [Cy, Cs])
            w3 = cp.tile([Cy, 27, Cy], BF16, tag="w3")
            nc.scalar.dma_start(w3[:], w3T.ap().rearrange("t i o -> i t o"))

            for rep in range(repeat):
                # ---- per-core data
                ybk = dp.tile([Cy, KC], BF16, tag="ybk")
                sk = dp.tile([Cs, KC], F32, tag="sk")
                yslab = dp.tile([Cy, 6, 18, 18], BF16, tag="yslab")
                # SP hwdge queue carries the latency-critical data path
                # (ybk + the y1 chunks); everything else rides the ACT queue.
                nc.sync.dma_start(ybk[:], Ybk.ap())
                nc.sync.dma_start(sk[:], Sk.ap())
                nc.scalar.dma_start(yslab[:], Yslab.ap())

                y1 = dp2.tile([Cs, N], BF16, tag="y1")
                kq = dp2.tile([Cs, KC], BF16, tag="kq")
                s1k = dp.tile([Cs, KC], BF16, tag="s1k")
                skb = dp.tile([Cs, KC], BF16, tag="skb")
                v = dp2.tile([128, KT, Cs], F32, tag="v")
                vp = dp2.tile([128, KT, Cs], FP8, tag="vp")
                e = dp2.tile([128, KT, N], FP8, tag="e")
                zr = dp.tile([128, KT], F32, tag="zr")
                z2 = dp.tile([128, KT, NQ], F32, tag="z2")

                qkv_on = 'qkv' not in ablate

                # ---- own-quarter chain first: y1k -> KQ, S1 -> V
                # (y1 relus run on ACT — idle until the exps and they gate the
                # exp cadence anyway; V/S1 chain and PSUM drains go to DVE)
                if qkv_on:
                    ps1 = pp.tile([Cs, QC], F32, tag="pss")
                    mm2(ps1[:], wy[:], ybk[:])
                    y1k = chp.tile([Cs, QC], BF16, tag="y1c")
                    nc.vector.tensor_scalar(y1k[:], ps1[:], bias[:, 0:1], 0.0,
                                            ADD, MAX)
                    ps2 = pp.tile([Cs, QC], F32, tag="pss")
                    mm2(ps2[:], wm[:], y1k[:])
                    nc.vector.tensor_copy(kq[:], ps2[:])
                    nc.vector.tensor_copy(skb[:], sk[:])
                    ps3 = pp.tile([Cs, QC], F32, tag="pss")
                    mm2(ps3[:], ws[:], skb[:])
                    nc.vector.tensor_scalar(s1k[:], ps3[:], bias[:, 1:2], 0.0,
                                            ADD, MAX)
                    for kt in range(KT):
                        psv = pp.tile([128, Cs], F32, tag="pss")
                        nc.tensor.matmul(psv[:], s1k[:, kt * 128:(kt + 1) * 128],
                                         wv[:], start=True, stop=True)
                        # 256x prescale: vp = 256*V/Z ~ 2e-2 sits inside the
                        # fp8e4m3 normal range (V/Z ~ 1e-4 would underflow);
                        # the xT PSUM drain divides it back out.
                        nc.vector.tensor_scalar_mul(v[:, kt, :], psv[:], 256.0)

                # ---- y1 for the full 4096 tokens (scores read y1 directly:
                # scoresT = kq^T y1 with kq = M^T y1k, M = scale Wq Wk^T)
                for ci in range(NQ if qkv_on else 0):
                    yc = chp.tile([Cy, QC], BF16, tag="yc")
                    nc.sync.dma_start(yc[:], Yb.ap()[:, ci * QC:(ci + 1) * QC])
                    ps1 = pp.tile([Cs, QC], F32, tag="pss")
                    mm2(ps1[:], wy[:], yc[:])
                    nc.vector.tensor_scalar(y1[:, ci * QC:(ci + 1) * QC], ps1[:],
                                            bias[:, 0:1], 0.0, ADD, MAX)

                # ---- attention: per-k-tile pipeline
                # scores -> exp(+Z accum) -> V' -> xT accumulation (query half 0)
                attn_on = 'attn' not in ablate and qkv_on
                conv_on = 'conv' not in ablate
                # PSUM budget (8x2KB banks): score double-buffer 2x(128,1024)
                # [4] + xT accumulator for query chunk 0 (48,1024) [2] + the
                # conv accumulator (96,1024) [2]. The conv's 54 taps run in
                # the PE slack of the ACT-bound exp phase (the RS is ~free on
                # real HW, so post-phase PE work is fully serial — keep it
                # minimal).
                xacc = pa.tile([Cs, QC], F32, tag="xacc")
                psc = pa.tile([Cy, QC], F32, tag="psc")
                if not attn_on:
                    nc.gpsimd.memset(v[:], 0.5)
                    nc.gpsimd.memset(e[:], 0.25)
                    nc.gpsimd.memset(vp[:], 0.5)
                # the xT matmuls for k-tile kt are emitted AFTER the scores of
                # kt+1 (one-tile software-pipeline delay) so the in-order PE
                # queue never stalls on V'(kt), which closes only with exp(kt).
                DR = mybir.MatmulPerfMode.DoubleRow

                def xt_mm(kt):
                    # kt is the odd index of a (kt-1, kt) pair; DoubleRow
                    # contracts both 128-key tiles in one pass at 0.5 cyc/row.
                    for h in range(2):
                        nc.tensor.matmul(
                            xacc[:, h * 512:(h + 1) * 512],
                            vp[:, kt - 1:kt + 1, :],
                            e[:, kt - 1:kt + 1, h * 512:(h + 1) * 512],
                            start=(kt == 1), stop=(kt == KT - 1), perf_mode=DR)

                def conv_taps(ts):
                    for t in ts:
                        kd, r = divmod(t, 9)
                        kh, kw = divmod(r, 3)
                        for h in range(2):
                            nc.tensor.matmul(
                                psc[:, h * 512:(h + 1) * 512], w3[:, t, :],
                                yslab[:, kd + 2 * h:kd + 2 * h + 2,
                                      kh:kh + 16, kw:kw + 16],
                                start=(t == 0), stop=(t == 26))

                # tap schedule: none in the first two k-tiles (yslab/w3 DMAs
                # still in flight), ~5 of the 27 taps per k-tile after
                bounds = [round(27 * i / 6) for i in range(7)]
                tap_sched = [[]] * 2 + [list(range(bounds[i], bounds[i + 1]))
                                        for i in range(6)]

                for kt in range(KT if attn_on else 0):
                    lhs = kq[:, kt * 128:(kt + 1) * 128]
                    for ci in range(NQ):
                        pss = pp.tile([128, QC], F32, tag="pss")
                        mm2(pss[:], lhs, y1[:, ci * QC:(ci + 1) * QC])
                        # accum_out gives the per-chunk Z partial for 187ns on
                        # ACT — far cheaper than a DVE row-reduce of E
                        nc.scalar.activation(
                            e[:, kt, ci * QC:(ci + 1) * QC], pss[:], AF.Exp,
                            accum_out=z2[:, kt, ci:ci + 1])
                    # Z(kt) closes with the last exp: 3 adds + recip (DVE)
                    nc.vector.tensor_add(zr[:, kt:kt + 1], z2[:, kt, 0:1],
                                         z2[:, kt, 1:2])
                    nc.vector.tensor_add(z2[:, kt, 2:3], z2[:, kt, 2:3],
                                         z2[:, kt, 3:4])
                    nc.vector.tensor_add(zr[:, kt:kt + 1], zr[:, kt:kt + 1],
                                         z2[:, kt, 2:3])
                    nc.vector.reciprocal(zr[:, kt:kt + 1], zr[:, kt:kt + 1])
                    nc.vector.tensor_scalar_mul(vp[:, kt, :], v[:, kt, :],
                                                zr[:, kt:kt + 1])
                    if kt % 2 == 1 and kt > 1:
                        xt_mm(kt - 2)
                    if conv_on:
                        conv_taps(tap_sched[kt])
                if attn_on:
                    xt_mm(KT - 1)
                if conv_on and not attn_on:
                    conv_taps(range(27))

                # ---- query half 1 xT + drains
                # query half 1 re-reads E from SBUF; the two accumulators live
                # in the (now idle) score-psum slots so they need not wait for
                # the half-0 drain.
                xc = dp.tile([Cs, N], BF16, tag="xc")
                if attn_on:
                    nc.vector.tensor_scalar_mul(xc[:, 0:QC], xacc[:],
                                                1.0 / 256.0)
                    for j in range(1, NQ):
                        x2 = pp.tile([Cs, QC], F32, tag="pss")
                        for kt in range(1, KT, 2):
                            for h in range(2):
                                nc.tensor.matmul(
                                    x2[:, h * 512:(h + 1) * 512],
                                    vp[:, kt - 1:kt + 1, :],
                                    e[:, kt - 1:kt + 1,
                                      j * QC + h * 512:j * QC + (h + 1) * 512],
                                    start=(kt == 1), stop=(kt == KT - 1),
                                    perf_mode=DR)
                        nc.vector.tensor_scalar_mul(
                            xc[:, j * QC:(j + 1) * QC], x2[:], 1.0 / 256.0)
                else:
                    nc.gpsimd.memset(xc[:], 0.3)

                # conv drain + Y2 projection (conv accumulated in-phase)
                c3 = dp.tile([Cy, KC], BF16, tag="c3")
                y2 = dp.tile([Cs, KC], F32, tag="y2")
                if conv_on:
                    nc.vector.tensor_copy(c3[:], psc[:])
                    psy = pp.tile([Cs, QC], F32, tag="pss")
                    mm2(psy[:], wy2[:], c3[:])
                    nc.vector.tensor_scalar(y2[:], psy[:], bias[:, 3:4], 0.0,
                                            ADD, MAX)
                    nc.sync.dma_start(OUT.ap()[Cs:2 * Cs, :], y2[:])

                # ---- ReduceScatter of the x^T partials
                cin = dram.tile([G * Cs, KC], BF16, tag="cin")
                cout = dram.tile([Cs, KC], BF16, tag="cout")
                for gg in range(G):
                    nc.sync.dma_start(cin[gg * Cs:(gg + 1) * Cs, :],
                                      xc[:, gg * KC:(gg + 1) * KC])
                if 'rs' not in ablate:
                    nc.gpsimd.collective_compute(
                        "ReduceScatter", mybir.AluOpType.add,
                        replica_groups=[[0, 1, 2, 3], [4, 5, 6, 7]],
                        ins=[cin[:]], outs=[cout[:]],
                    )
                else:
                    nc.sync.dma_start(cout[:], cin[0:Cs, :])

                # ---- post-RS: out-projection, mul by S+pe
                xr = dp.tile([Cs, KC], BF16, tag="xr")
                nc.sync.dma_start(xr[:], cout[:])
                zc = dp.tile([Cs, KC], F32, tag="zc")
                zo = dp.tile([Cs, KC], F32, tag="zo")
                psz = pp.tile([Cs, QC], F32, tag="pss")
                mm2(psz[:], wo[:], xr[:])
                nc.vector.tensor_scalar(zc[:], psz[:], bias[:, 2:3], 0.0,
                                        ADD, MAX)
                nc.vector.tensor_mul(zo[:], zc[:], sk[:])
                nc.sync.dma_start(OUT.ap()[0:Cs, :], zo[:])

    nc.compile()
    return nc


class _Runner:
    """Builds the bass module once and a single reusable jitted callable
    (re-jitting per call would re-trace + re-hash the BIR module: ~600ms)."""

    def __init__(self, repeat=1, ablate=(), **kw):
        install_neuronx_cc_hook()
        nc = _build(repeat, ablate, **kw)
        self._setup_from_nc(nc)

    def _setup_from_nc(self, nc):
        install_neuronx_cc_hook()
        pid = nc.partition_id_tensor.name if nc.partition_id_tensor else None
        in_names, out_names, out_avals = [], [], []
        for alloc in nc.m.functions[0].allocations:
            if not isinstance(alloc, mybir.MemoryLocationSet):
                continue
            name = alloc.memorylocations[0].name
            if alloc.kind == "ExternalInput":
                if name != pid:
                    in_names.append(name)
            elif alloc.kind == "ExternalOutput":
                out_names.append(name)
                out_avals.append(jax.core.ShapedArray(
                    tuple(alloc.tensor_shape), mybir.dt.np(alloc.dtype)))
        self.in_names, self.out_names, self.out_avals = in_names, out_names, out_avals
        all_names = in_names + out_names + ([pid] if pid else [])

        def _body(*args):
            operands = list(args)
            if pid is not None:
                operands.append(bass2jax.partition_id_tensor())
            return tuple(_bass_exec_p.bind(
                *operands, out_avals=tuple(out_avals), in_names=tuple(all_names),
                out_names=tuple(out_names), lowering_input_output_aliases=(),
                sim_require_finite=True, sim_require_nnan=True, nc=nc))

        mesh = self.mesh = Mesh(np.asarray(jax.devices()[:NC]), ("core",))
        sp = (PartitionSpec("core"),)
        n_in = len(in_names) + len(out_names)
        self.fn = jax.jit(
            shard_map(_body, mesh=mesh, in_specs=sp * n_in,
                      out_specs=sp * len(out_names), check_rep=False),
            keep_unused=True)

    def device_args(self, in_maps):
        """Pre-stage all inputs on device (sharded) for low-overhead timed calls."""
        from jax.sharding import NamedSharding
        sh = NamedSharding(self.mesh, PartitionSpec("core"))
        cat = [np.concatenate([in_maps[c][n] for c in range(NC)], axis=0)
               for n in self.in_names]
        zz = [np.zeros((NC * a.shape[0], *a.shape[1:]), a.dtype)
              for a in self.out_avals]
        return [jax.device_put(a, sh) for a in cat + zz]

    def __call__(self, in_maps):
        outs = self.fn(*self.device_args(in_maps))
        jax.block_until_ready(outs)
        return [
            {n: np.asarray(outs[i]).reshape(NC, *self.out_avals[i].shape)[c]
             for i, n in enumerate(self.out_names)}
            for c in range(NC)
        ]


def _get(repeat=1, ablate=(), **kw):
    key = (repeat, tuple(sorted(ablate)), tuple(sorted(kw.items())))
    if key not in _cache:
        _cache[key] = _Runner(repeat, ablate, **kw)
    return _cache[key]


# ---------------------------------------------------------------- entry point
def kernel(**inputs):
    in_maps = _prepare(inputs)
    results = _get(1)(in_maps)
    out = np.zeros((B, 2 * Cs, D, H, W), np.float32)
    for core in range(NC):
        b, g = divmod(core, G)
        blk = results[core]["OUT"].reshape(2 * Cs, D // G, H, W)
        out[b, :, g * (D // G):(g + 1) * (D // G)] = blk
    return out


# revision 37
# speedup vs baseline: 1.4016x; 1.4016x over previous
"""Trainium2 Bass kernel for nn_MultiHeadCrossAttention_57638461112647.

Sharding: 8 cores = 2 batches x 4-way split over attention *keys* (and,
identically, over output tokens). The softmax in the reference is over the
*query* axis (axis=1), so with scores laid out (keys on partitions, queries on
free) the softmax denominator Z[k] is a free-axis row-sum, fully core-local.
The only cross-core exchange is one ReduceScatter of the attention output
partials x^T = V'^T E (48 x 4096) within each 4-core batch group.

Structure (bf16 operands everywhere, fp32 PSUM accumulation; matmuls split to
the 512-element ISA moving limit):
  - scoresT = (M^T y1k)^T y1 with M = scale*Wq@Wk^T folded on host: Q and K
    never materialize on device, scores read the relu'd y1 directly.
  - Per-k-tile software pipeline: scores(kt) [PE] -> exp(kt) [ACT] -> Z(kt)
    row-sum [DVE] -> V'(kt)=V/Z [DVE] -> xT(kt) accumulated in PSUM. Z is
    per-KEY (row sum over the free/query axis), so each k-tile's Z closes as
    soon as its exps retire; the xT and conv matmuls ride in the PE slack of
    the ACT-bound exp phase (the xT/conv emission trails the scores by one
    k-tile so the in-order PE queue never stalls on V').
  - PSUM budget (8x2KB banks): score double-buffer 2x(128,1024) [4] + xT
    accumulator for query chunk 0 (48,1024) [2] + conv accumulator (96,1024)
    [2]. Query chunks 1-3 of xT re-read E from SBUF right after the phase.
  - The last k-tile's Z is computed as 4 per-chunk partial reduces so it
    closes right behind the final exp instead of a full-row reduce.
  - The ReduceScatter measures ~free on this hardware (intra-chip 4-rank
    groups); only the small out-projection tail follows it.
  - Cross-rep pipelining: e/y1/kq/v/vp double-buffered so consecutive repeat
    bodies overlap (the benchmark measures steady-state initiation interval,
    which sits at the PE busy floor of ~43us/rep).
  - Own-quarter reuse: KQ/V/S1 derive from a pre-sliced own-chunk input.

Host-side prep (cheap): fold BN-as-affine + biases into the 1x1-conv weights,
fold the attention scale into Wq, fold the 3x3x3-conv bias through the
following 1x1 conv, add the (constant) 3D positional encodings, pad the conv
input with its halo, bf16-round everything, and slice per-core chunks.
"""
import numpy as np
import ml_dtypes
import jax
from jax.sharding import Mesh, PartitionSpec
from jax.experimental.shard_map import shard_map

import concourse.bass as bass
import concourse.mybir as mybir
import concourse.tile as tile
from concourse import bacc
from concourse import bass2jax
from concourse.bass2jax import _bass_exec_p, install_neuronx_cc_hook

F32 = mybir.dt.float32
BF16 = mybir.dt.bfloat16
FP8 = mybir.dt.float8e4
AF = mybir.ActivationFunctionType
ADD = mybir.AluOpType.add
MAX = mybir.AluOpType.max

B, Cy, Cs, D, H, W = 2, 96, 48, 16, 16, 16
N = D * H * W            # 4096 tokens
NC = 8                   # cores
G = 4                    # cores per batch
KC = N // G              # keys / output tokens per core = 1024
KT = KC // 128           # k-tiles per core = 8
QC = 1024                # free-dim chunk per bf16 matmul
NQ = N // QC             # query chunks = 4
EPS = 1e-5

_cache = {}


# ---------------------------------------------------------------- host prep
def _pe3d(C, x, y, z):
    """Transcription of reference.pe3d (incl. its quirky torch broadcasting)."""
    c = int(np.ceil(C / 3))
    inv_freq = (1.0 / (10000.0 ** (np.arange(0, c, 2, dtype=np.float32) / c))
                ).astype(np.float32)

    def emb(n):
        s = np.arange(n, dtype=np.float32)[:, None] * inv_freq[None, :]
        return np.concatenate([np.sin(s), np.cos(s)], axis=-1).astype(np.float32)

    out = np.zeros((x, y, z, 3 * c), np.float32)
    out[..., :c] = emb(x)[:, None, :]        # broadcasts against (y, z, c)
    out[..., c:2 * c] = emb(y)[:, None, :]
    out[..., 2 * c:3 * c] = emb(z)
    return np.ascontiguousarray(out[..., :C].transpose(3, 0, 1, 2))  # (C,x,y,z)


def _prepare(inputs):
    f = lambda a: np.ascontiguousarray(np.asarray(a, np.float32))
    bf = lambda a: np.ascontiguousarray(np.asarray(a, ml_dtypes.bfloat16))
    Y, S = f(inputs['Y']), f(inputs['S'])

    pe_s = _pe3d(Cs, D, H, W).reshape(Cs, N)
    pe_y = _pe3d(Cy, D, H, W).reshape(Cy, N)
    Scm = S.reshape(B, Cs, N) + pe_s[None]
    Ycm = Y.reshape(B, Cy, N) + pe_y[None]

    sb = lambda g: f(g) / np.sqrt(np.float32(1.0) + np.float32(EPS))

    def fold(w, b, g, be):
        s = sb(g)
        return f(w) * s[:, None], (f(b) * s + f(be)).astype(np.float32)

    WsF, bsF = fold(inputs['w_s'], inputs['b_s'], inputs['g_s'], inputs['be_s'])
    WyF, byF = fold(inputs['w_y'], inputs['b_y'], inputs['g_y'], inputs['be_y'])
    WoF, boF = fold(inputs['w_o'], inputs['b_o'], inputs['g_o'], inputs['be_o'])
    Wy2F, by2F = fold(inputs['w_y2'], inputs['b_y2'], inputs['g_y2'], inputs['be_y2'])
    by2FF = (Wy2F @ f(inputs['b3']) + by2F).astype(np.float32)

    c = np.ascontiguousarray
    # scoresT = (M^T y1k)^T y1 with M = scale * Wq Wk^T: Q and K never
    # materialize on device.
    M = bf(np.float32(Cs) ** np.float32(-0.5) * f(inputs['Wq']) @ f(inputs['Wk']).T)
    Wv = bf(inputs['Wv'])
    w3T = bf(f(inputs['w3']).reshape(Cy, Cy, 27).transpose(2, 1, 0))  # (27,96,96)
    BIAS = c(np.stack([byF, bsF, boF, by2FF], axis=1))                # (48,4)

    Ypad = np.zeros((B, Cy, D + 2, H + 2, W + 2), np.float32)
    Ypad[:, :, 1:-1, 1:-1, 1:-1] = Ycm.reshape(B, Cy, D, H, W)

    in_maps = []
    for core in range(NC):
        b, g = divmod(core, G)
        d0 = g * (D // G)
        in_maps.append(dict(
            Yb=bf(Ycm[b]),                                 # (96,4096) bf16
            Ybk=bf(Ycm[b, :, g * KC:(g + 1) * KC]),        # (96,1024) bf16
            Sk=c(Scm[b, :, g * KC:(g + 1) * KC]),          # (48,1024) f32
            Yslab=bf(Ypad[b, :, d0:d0 + 6, :, :]),         # (96,6,18,18) bf16
            WsT=bf(WsF.T), WyT=bf(WyF.T), WoT=bf(WoF.T), Wy2T=bf(Wy2F.T),
            M=M, Wv=Wv, w3T=w3T, BIAS=BIAS,
        ))
    return in_maps


# ---------------------------------------------------------------- bass build
def _build(repeat=1, ablate=()):
    """ablate: subset of {'rs','attn','conv','qkv'} — for timing bisection
    only (results become wrong)."""
    nc = bacc.Bacc("TRN2", target_bir_lowering=False, debug=False, num_devices=NC)

    Yb = nc.dram_tensor("Yb", [Cy, N], BF16, kind="ExternalInput")
    Sk = nc.dram_tensor("Sk", [Cs, KC], F32, kind="ExternalInput")
    Yslab = nc.dram_tensor("Yslab", [Cy, 6, 18, 18], BF16, kind="ExternalInput")
    WsT = nc.dram_tensor("WsT", [Cs, Cs], BF16, kind="ExternalInput")
    WyT = nc.dram_tensor("WyT", [Cy, Cs], BF16, kind="ExternalInput")
    WoT = nc.dram_tensor("WoT", [Cs, Cs], BF16, kind="ExternalInput")
    Wy2T = nc.dram_tensor("Wy2T", [Cy, Cs], BF16, kind="ExternalInput")
    M = nc.dram_tensor("M", [Cs, Cs], BF16, kind="ExternalInput")
    Wv = nc.dram_tensor("Wv", [Cs, Cs], BF16, kind="ExternalInput")
    w3T = nc.dram_tensor("w3T", [27, Cy, Cy], BF16, kind="ExternalInput")
    BIAS = nc.dram_tensor("BIAS", [Cs, 4], F32, kind="ExternalInput")
    OUT = nc.dram_tensor("OUT", [2 * Cs, KC], F32, kind="ExternalOutput")

    # The NEFF is identical across cores, so the own-chunk index g cannot be
    # baked in; the host passes the own-quarter slice as a separate input.
    Ybk = nc.dram_tensor("Ybk", [Cy, KC], BF16, kind="ExternalInput")

    def mm2(out, lhsT, rhs, start=True, stop=True):
        """matmul split into <=512-wide halves (ISA moving-operand limit)."""
        n = out.shape[-1]
        if n <= 512:
            nc.tensor.matmul(out, lhsT, rhs, start=start, stop=stop)
            return
        h = n // 2
        nc.tensor.matmul(out[:, 0:h], lhsT, rhs[:, 0:h], start=start, stop=stop)
        nc.tensor.matmul(out[:, h:n], lhsT, rhs[:, h:n], start=start, stop=stop)

    with tile.TileContext(nc) as tc:
        with (
            tc.tile_pool(name="const", bufs=1) as cp,
            tc.tile_pool(name="data", bufs=1) as dp,
            tc.tile_pool(name="data2", bufs=2) as dp2,
            tc.tile_pool(name="chunk", bufs=2) as chp,
            tc.tile_pool(name="psum", bufs=2, space="PSUM") as pp,
            tc.tile_pool(name="psacc", bufs=1, space="PSUM") as pa,
            tc.tile_pool(name="dram", bufs=1, space="DRAM") as dram,
        ):
            # ---- constants into SBUF
            def load_const(t, shape, dt=BF16):
                s = cp.tile(shape, dt, tag=t.name)
                nc.scalar.dma_start(s[:], t.ap())
                return s

            wy = load_const(WyT, [Cy, Cs])
            wm = load_const(M, [Cs, Cs])
            ws = load_const(WsT, [Cs, Cs])
            wv = load_const(Wv, [Cs, Cs])
            bias = load_const(BIAS, [Cs, 4], F32)
            wo = load_const(WoT, [Cs, Cs])
            wy2 = load_const(Wy2T, [Cy, Cs])
            w3 = cp.tile([Cy, 27, Cy], BF16, tag="w3")
            nc.scalar.dma_start(w3[:], w3T.ap().rearrange("t i o -> i t o"))

            for rep in range(repeat):
                # ---- per-core data
                ybk = dp.tile([Cy, KC], BF16, tag="ybk")
                sk = dp.tile([Cs, KC], F32, tag="sk")
                yslab = dp.tile([Cy, 6, 18, 18], BF16, tag="yslab")
                # SP hwdge queue carries the latency-critical data path
                # (ybk + the y1 chunks); everything else rides the ACT queue.
                nc.sync.dma_start(ybk[:], Ybk.ap())
                nc.sync.dma_start(sk[:], Sk.ap())
                nc.scalar.dma_start(yslab[:], Yslab.ap())

                y1 = dp2.tile([Cs, N], BF16, tag="y1")
                kq = dp2.tile([Cs, KC], BF16, tag="kq")
                s1k = dp.tile([Cs, KC], BF16, tag="s1k")
                skb = dp.tile([Cs, KC], BF16, tag="skb")
                v = dp2.tile([128, KT, Cs], F32, tag="v")
                vp = dp2.tile([128, KT, Cs], FP8, tag="vp")
                e = dp2.tile([128, KT, N], FP8, tag="e")
                zr = dp.tile([128, KT], F32, tag="zr")

                qkv_on = 'qkv' not in ablate

                # ---- own-quarter chain first: y1k -> KQ, S1 -> V
                # (y1 relus run on ACT — idle until the exps and they gate the
                # exp cadence anyway; V/S1 chain and PSUM drains go to DVE)
                if qkv_on:
                    ps1 = pp.tile([Cs, QC], F32, tag="pss")
                    mm2(ps1[:], wy[:], ybk[:])
                    y1k = chp.tile([Cs, QC], BF16, tag="y1c")
                    nc.scalar.activation(y1k[:], ps1[:], AF.Relu, bias=bias[:, 0:1])
                    ps2 = pp.tile([Cs, QC], F32, tag="pss")
                    mm2(ps2[:], wm[:], y1k[:])
                    nc.vector.tensor_copy(kq[:], ps2[:])
                    nc.vector.tensor_copy(skb[:], sk[:])
                    ps3 = pp.tile([Cs, QC], F32, tag="pss")
                    mm2(ps3[:], ws[:], skb[:])
                    nc.vector.tensor_scalar(s1k[:], ps3[:], bias[:, 1:2], 0.0,
                                            ADD, MAX)
                    for kt in range(KT):
                        psv = pp.tile([128, Cs], F32, tag="pss")
                        nc.tensor.matmul(psv[:], s1k[:, kt * 128:(kt + 1) * 128],
                                         wv[:], start=True, stop=True)
                        # 256x prescale: vp = 256*V/Z ~ 2e-2 sits inside the
                        # fp8e4m3 normal range (V/Z ~ 1e-4 would underflow);
                        # the xT PSUM drain divides it back out.
                        nc.vector.tensor_scalar_mul(v[:, kt, :], psv[:], 256.0)

                # ---- y1 for the full 4096 tokens (scores read y1 directly:
                # scoresT = kq^T y1 with kq = M^T y1k, M = scale Wq Wk^T)
                for ci in range(NQ if qkv_on else 0):
                    yc = chp.tile([Cy, QC], BF16, tag="yc")
                    nc.sync.dma_start(yc[:], Yb.ap()[:, ci * QC:(ci + 1) * QC])
                    ps1 = pp.tile([Cs, QC], F32, tag="pss")
                    mm2(ps1[:], wy[:], yc[:])
                    nc.scalar.activation(y1[:, ci * QC:(ci + 1) * QC], ps1[:],
                                         AF.Relu, bias=bias[:, 0:1])

                # ---- attention: per-k-tile pipeline
                # scores -> exp(+Z accum) -> V' -> xT accumulation (query half 0)
                attn_on = 'attn' not in ablate and qkv_on
                conv_on = 'conv' not in ablate
                # PSUM budget (8x2KB banks): score double-buffer 2x(128,1024)
                # [4] + xT accumulator for query chunk 0 (48,1024) [2] + the
                # conv accumulator (96,1024) [2]. The conv's 54 taps run in
                # the PE slack of the ACT-bound exp phase (the RS is ~free on
                # real HW, so post-phase PE work is fully serial — keep it
                # minimal).
                xacc = pa.tile([Cs, QC], F32, tag="xacc")
                psc = pa.tile([Cy, QC], F32, tag="psc")
                if not attn_on:
                    nc.gpsimd.memset(v[:], 0.5)
                    nc.gpsimd.memset(e[:], 0.25)
                    nc.gpsimd.memset(vp[:], 0.5)
                # the xT matmuls for k-tile kt are emitted AFTER the scores of
                # kt+1 (one-tile software-pipeline delay) so the in-order PE
                # queue never stalls on V'(kt), which closes only with exp(kt).
                DR = mybir.MatmulPerfMode.DoubleRow

                def xt_mm(kt):
                    # kt is the odd index of a (kt-1, kt) pair; DoubleRow
                    # contracts both 128-key tiles in one pass at 0.5 cyc/row.
                    for h in range(2):
                        nc.tensor.matmul(
                            xacc[:, h * 512:(h + 1) * 512],
                            vp[:, kt - 1:kt + 1, :],
                            e[:, kt - 1:kt + 1, h * 512:(h + 1) * 512],
                            start=(kt == 1), stop=(kt == KT - 1), perf_mode=DR)

                def conv_taps(ts):
                    for t in ts:
                        kd, r = divmod(t, 9)
                        kh, kw = divmod(r, 3)
                        for h in range(2):
                            nc.tensor.matmul(
                                psc[:, h * 512:(h + 1) * 512], w3[:, t, :],
                                yslab[:, kd + 2 * h:kd + 2 * h + 2,
                                      kh:kh + 16, kw:kw + 16],
                                start=(t == 0), stop=(t == 26))

                # tap schedule: none in the first two k-tiles (yslab/w3 DMAs
                # still in flight), ~5 of the 27 taps per k-tile after
                bounds = [round(27 * i / 6) for i in range(7)]
                tap_sched = [[]] * 2 + [list(range(bounds[i], bounds[i + 1]))
                                        for i in range(6)]

                z4 = dp.tile([128, NQ], F32, tag="z4")
                for kt in range(KT if attn_on else 0):
                    lhs = kq[:, kt * 128:(kt + 1) * 128]
                    last = kt == KT - 1
                    for ci in range(NQ):
                        pss = pp.tile([128, QC], F32, tag="pss")
                        mm2(pss[:], lhs, y1[:, ci * QC:(ci + 1) * QC])
                        nc.scalar.activation(
                            e[:, kt, ci * QC:(ci + 1) * QC], pss[:], AF.Exp)
                        if last:
                            # split the final tile's Z into per-chunk partials
                            # so it closes right after the last exp instead of
                            # costing a full-row reduce on the critical tail
                            nc.vector.tensor_reduce(
                                z4[:, ci:ci + 1],
                                e[:, kt, ci * QC:(ci + 1) * QC],
                                mybir.AxisListType.X, ADD)
                    # Z(kt) closes here: row-sum of E + reciprocal + scale V
                    # (DVE, off the ACT/PE critical path)
                    if last:
                        nc.vector.tensor_add(zr[:, kt:kt + 1], z4[:, 0:1],
                                             z4[:, 1:2])
                        nc.vector.tensor_add(z4[:, 2:3], z4[:, 2:3], z4[:, 3:4])
                        nc.vector.tensor_add(zr[:, kt:kt + 1], zr[:, kt:kt + 1],
                                             z4[:, 2:3])
                    else:
                        nc.vector.tensor_reduce(zr[:, kt:kt + 1], e[:, kt, :],
                                                mybir.AxisListType.X, ADD)
                    nc.vector.reciprocal(zr[:, kt:kt + 1], zr[:, kt:kt + 1])
                    nc.vector.tensor_scalar_mul(vp[:, kt, :], v[:, kt, :],
                                                zr[:, kt:kt + 1])
                    if kt % 2 == 1 and kt > 1:
                        xt_mm(kt - 2)
                    if conv_on:
                        conv_taps(tap_sched[kt])
                if attn_on:
                    xt_mm(KT - 1)
                if conv_on and not attn_on:
                    conv_taps(range(27))

                # ---- query half 1 xT + drains
                # query half 1 re-reads E from SBUF; the two accumulators live
                # in the (now idle) score-psum slots so they need not wait for
                # the half-0 drain.
                xc = dp.tile([Cs, N], BF16, tag="xc")
                if attn_on:
                    nc.scalar.activation(xc[:, 0:QC], xacc[:], AF.Copy,
                                         scale=1.0 / 256.0)
                    for j in range(1, NQ):
                        x2 = pp.tile([Cs, QC], F32, tag="pss")
                        for kt in range(1, KT, 2):
                            for h in range(2):
                                nc.tensor.matmul(
                                    x2[:, h * 512:(h + 1) * 512],
                                    vp[:, kt - 1:kt + 1, :],
                                    e[:, kt - 1:kt + 1,
                                      j * QC + h * 512:j * QC + (h + 1) * 512],
                                    start=(kt == 1), stop=(kt == KT - 1),
                                    perf_mode=DR)
                        nc.scalar.activation(xc[:, j * QC:(j + 1) * QC], x2[:],
                                             AF.Copy, scale=1.0 / 256.0)
                else:
                    nc.gpsimd.memset(xc[:], 0.3)

                # conv drain + Y2 projection (conv accumulated in-phase)
                c3 = dp.tile([Cy, KC], BF16, tag="c3")
                y2 = dp.tile([Cs, KC], F32, tag="y2")
                if conv_on:
                    nc.vector.tensor_copy(c3[:], psc[:])
                    psy = pp.tile([Cs, QC], F32, tag="pss")
                    mm2(psy[:], wy2[:], c3[:])
                    nc.vector.tensor_scalar(y2[:], psy[:], bias[:, 3:4], 0.0,
                                            ADD, MAX)
                    nc.sync.dma_start(OUT.ap()[Cs:2 * Cs, :], y2[:])

                # ---- ReduceScatter of the x^T partials
                cin = dram.tile([G * Cs, KC], BF16, tag="cin")
                cout = dram.tile([Cs, KC], BF16, tag="cout")
                for gg in range(G):
                    nc.sync.dma_start(cin[gg * Cs:(gg + 1) * Cs, :],
                                      xc[:, gg * KC:(gg + 1) * KC])
                if 'rs' not in ablate:
                    nc.gpsimd.collective_compute(
                        "ReduceScatter", mybir.AluOpType.add,
                        replica_groups=[[0, 1, 2, 3], [4, 5, 6, 7]],
                        ins=[cin[:]], outs=[cout[:]],
                    )
                else:
                    nc.sync.dma_start(cout[:], cin[0:Cs, :])

                # ---- post-RS: out-projection, mul by S+pe
                xr = dp.tile([Cs, KC], BF16, tag="xr")
                nc.sync.dma_start(xr[:], cout[:])
                zc = dp.tile([Cs, KC], F32, tag="zc")
                zo = dp.tile([Cs, KC], F32, tag="zo")
                psz = pp.tile([Cs, QC], F32, tag="pss")
                mm2(psz[:], wo[:], xr[:])
                nc.scalar.activation(zc[:], psz[:], AF.Relu, bias=bias[:, 2:3])
                nc.vector.tensor_mul(zo[:], zc[:], sk[:])
                nc.sync.dma_start(OUT.ap()[0:Cs, :], zo[:])

    nc.compile()
    return nc


class _Runner:
    """Builds the bass module once and a single reusable jitted callable
    (re-jitting per call would re-trace + re-hash the BIR module: ~600ms)."""

    def __init__(self, repeat=1, ablate=(), **kw):
        install_neuronx_cc_hook()
        nc = _build(repeat, ablate, **kw)
        self._setup_from_nc(nc)

    def _setup_from_nc(self, nc):
        install_neuronx_cc_hook()
        pid = nc.partition_id_tensor.name if nc.partition_id_tensor else None
        in_names, out_names, out_avals = [], [], []
        for alloc in nc.m.functions[0].allocations:
            if not isinstance(alloc, mybir.MemoryLocationSet):
                continue
            name = alloc.memorylocations[0].name
            if alloc.kind == "ExternalInput":
                if name != pid:
                    in_names.append(name)
            elif alloc.kind == "ExternalOutput":
                out_names.append(name)
                out_avals.append(jax.core.ShapedArray(
                    tuple(alloc.tensor_shape), mybir.dt.np(alloc.dtype)))
        self.in_names, self.out_names, self.out_avals = in_names, out_names, out_avals
        all_names = in_names + out_names + ([pid] if pid else [])

        def _body(*args):
            operands = list(args)
            if pid is not None:
                operands.append(bass2jax.partition_id_tensor())
            return tuple(_bass_exec_p.bind(
                *operands, out_avals=tuple(out_avals), in_names=tuple(all_names),
                out_names=tuple(out_names), lowering_input_output_aliases=(),
                sim_require_finite=True, sim_require_nnan=True, nc=nc))

        mesh = self.mesh = Mesh(np.asarray(jax.devices()[:NC]), ("core",))
        sp = (PartitionSpec("core"),)
        n_in = len(in_names) + len(out_names)
        self.fn = jax.jit(
            shard_map(_body, mesh=mesh, in_specs=sp * n_in,
                      out_specs=sp * len(out_names), check_rep=False),
            keep_unused=True)

    def device_args(self, in_maps):
        """Pre-stage all inputs on device (sharded) for low-overhead timed calls."""
        from jax.sharding import NamedSharding
        sh = NamedSharding(self.mesh, PartitionSpec("core"))
        cat = [np.concatenate([in_maps[c][n] for c in range(NC)], axis=0)
               for n in self.in_names]
        zz = [np.zeros((NC * a.shape[0], *a.shape[1:]), a.dtype)
              for a in self.out_avals]
        return [jax.device_put(a, sh) for a in cat + zz]

    def __call__(self, in_maps):
        outs = self.fn(*self.device_args(in_maps))
        jax.block_until_ready(outs)
        return [
            {n: np.asarray(outs[i]).reshape(NC, *self.out_avals[i].shape)[c]
             for i, n in enumerate(self.out_names)}
            for c in range(NC)
        ]


def _get(repeat=1, ablate=(), **kw):
    key = (repeat, tuple(sorted(ablate)), tuple(sorted(kw.items())))
    if key not in _cache:
        _cache[key] = _Runner(repeat, ablate, **kw)
    return _cache[key]


# ---------------------------------------------------------------- entry point
def kernel(**inputs):
    in_maps = _prepare(inputs)
    results = _get(1)(in_maps)
    out = np.zeros((B, 2 * Cs, D, H, W), np.float32)
    for core in range(NC):
        b, g = divmod(core, G)
        blk = results[core]["OUT"].reshape(2 * Cs, D // G, H, W)
        out[b, :, g * (D // G):(g + 1) * (D // G)] = blk
    return out
